# revision 16
# baseline (speedup 1.0000x reference)
"""Causal self-attention on 8 TRN2 NeuronCores (Bass/Tile, SPMD).

Problem: y = CausalSelfAttention(x; Wqkv, bqkv, Wproj, bproj)
  x [B=4, T=2048, C=1024], H=16 heads, D=64.

Sharding: core c = (batch b = c//2, head-half hh = c%2). Each core computes
q/k/v for its 8 heads of its batch (Wqkv column-sharded), full causal
attention for those heads, and a partial output projection (Wproj
row-sharded). Host sums the two bf16 partials per batch and adds bproj.

Schedule (v2): the PE is the bottleneck (~225us of streamed columns), so
every phase interleaves "fill" matmuls into the ACT-paced attention loop:
  - hp0 attention is interleaved with v-projection tiles and the next
    q-block's q/k projection; hp1/hp2 with the next head-pair's q/k.
  - hp3 attention is interleaved with the *output projection* of the
    previous q-block (its softmax denominators are normalized per-q-block
    instead of per-head-pair), so only the last q-block's projection
    remains as a serial tail.
  - Input DMAs are spread across the sync/scalar/vector queues, with a
    duplicated first-q-block slice of x (x0) so the first q/k projection
    starts before the full x transfer lands.
  - Both heads' AV accumulate into one [65, 1024] PSUM tile, halving the
    epilogue copies/DMAs; causal mask multiplies run on GpSimd.
"""

import math
from contextlib import ExitStack

import numpy as np
import ml_dtypes

import concourse.tile as tile
from concourse import bacc, mybir

BF16 = mybir.dt.bfloat16
F32 = mybir.dt.float32
NPBF16 = ml_dtypes.bfloat16

P = 128  # partitions / k-tile size
QB = 512  # q-block (matmul N; one fp32 PSUM bank)

B, T, C, H, D = 4, 2048, 1024, 16, 64
N_CORES = 8
HL = H // (N_CORES // B)  # heads per core (8)
CL = HL * D  # local head width (512)

# ---------------------------------------------------------------------------
# Per-core Bass program
# ---------------------------------------------------------------------------


def build_kernel(T=T, C=C, HL=HL, D=D, Cout=C):
    CL = HL * D
    n_ct = C // P
    n_mt = CL // P
    n_tt = T // P
    n_qb = T // QB
    n_hp = HL // 2
    dpb = QB // P
    n_cb = Cout // QB
    scale = 1.0 / math.sqrt(D)
    D1 = D + 1
    ca = (n_ct + 1) // 2  # x c-tiles in chunk a (rest in chunk b)
    cb_n = n_ct - ca

    assert C % P == 0 and CL % P == 0 and T % QB == 0 and Cout % QB == 0
    assert HL % 2 == 0 and D == 64 and n_mt == n_hp

    nc = bacc.Bacc("TRN2", target_bir_lowering=False, debug=False)
    # x first q-block, half-packed for 2KB DMA rows: row r = [x[r] | x[C/2+r]]
    x0T = nc.dram_tensor("x0T", [C // 2, 2 * QB], BF16, kind="ExternalInput")
    xT = nc.dram_tensor("xT", [C, T], BF16, kind="ExternalInput")
    # per-hp k|q chunks, half-packed into 1KB rows the same way
    wqk = nc.dram_tensor("wqk", [n_hp, C // 2, 4 * P], BF16, kind="ExternalInput")
    wv = nc.dram_tensor("wv", [C, CL], BF16, kind="ExternalInput")
    wp = nc.dram_tensor("wp", [CL, Cout], BF16, kind="ExternalInput")
    masks = nc.dram_tensor("masks", [P, P], BF16, kind="ExternalInput")
    out = nc.dram_tensor("out", [T, Cout], BF16, kind="ExternalOutput")

    with tile.TileContext(nc) as tc, ExitStack() as ctx:
        persist = ctx.enter_context(tc.tile_pool(name="persist", bufs=1))
        # PSUM budget (8 banks): scores 2x[128,1024] + AV 1x[65,1024] + u 2x[128,512]
        ps_sc = ctx.enter_context(tc.tile_pool(name="ps_sc", bufs=2, space="PSUM"))
        ps_av = ctx.enter_context(tc.tile_pool(name="ps_av", bufs=1, space="PSUM"))
        ps_u = ctx.enter_context(tc.tile_pool(name="ps_u", bufs=2, space="PSUM"))
        ppool = ctx.enter_context(tc.tile_pool(name="ppool", bufs=6))
        spool = ctx.enter_context(tc.tile_pool(name="spool", bufs=3))
        bcpool = ctx.enter_context(tc.tile_pool(name="bcpool", bufs=6))
        stage = ctx.enter_context(tc.tile_pool(name="stage", bufs=3))
        opool = ctx.enter_context(tc.tile_pool(name="opool", bufs=2))
        dram = ctx.enter_context(tc.tile_pool(name="dram", bufs=1, space="DRAM"))

        # ---- persistent input tiles; fine-grained DMAs for pipelining,
        # spread over the scalar and sync queues by first-use time ----
        x0_sb = persist.tile([P, n_ct, QB], BF16, tag="x0", name="x0")
        TH = T // 2
        xh_sb = [
            [
                persist.tile([P, TH], BF16, tag=f"x{c}_{h}", name=f"x{c}_{h}")
                for h in range(2)
            ]
            for c in range(n_ct)
        ]
        wqk_sb = persist.tile([P, n_hp, n_ct, 2, P], BF16, tag="wqk", name="wqk")
        wv_sb = [
            persist.tile([P, CL], BF16, tag=f"wv{c}", name=f"wv{c}")
            for c in range(n_ct)
        ]
        wp_sb = persist.tile([P, n_mt, Cout], BF16, tag="wp", name="wp")
        trimask = persist.tile([P, P], BF16, tag="trimask", name="trimask")

        hct = n_ct // 2

        def dma_wqk(hp):
            for h in range(2):
                nc.sync.dma_start(
                    wqk_sb[:][:, hp, hct * h : hct * (h + 1), :, :],
                    wqk[hp][:, h * 2 * P : (h + 1) * 2 * P].rearrange(
                        "(i p) (t w) -> p i t w", p=P, w=P
                    ),
                )

        # scalar queue: x0 halves first (unblock qk0+v0), then x first halves
        for h in range(2):
            nc.scalar.dma_start(
                x0_sb[:][:, hct * h : hct * (h + 1), :],
                x0T[:, h * QB : (h + 1) * QB].rearrange("(i p) c -> p i c", p=P),
            )
        for c in range(n_ct):
            nc.scalar.dma_start(xh_sb[c][0][:], xT[c * P : (c + 1) * P, 0:TH])
        # sync queue: hp0 q/k weights and wv in the first in-flight window,
        # bulk behind (the ~6-deep DGE window throttles it naturally)
        dma_wqk(0)
        for c in range(n_ct):
            nc.sync.dma_start(wv_sb[c][:], wv[c * P : (c + 1) * P, :])
        for c in range(n_ct):
            nc.sync.dma_start(xh_sb[c][1][:], xT[c * P : (c + 1) * P, TH:])
        nc.sync.dma_start(trimask[:], masks[:])
        for hp in range(1, n_hp):
            dma_wqk(hp)
        nc.sync.dma_start(wp_sb[:], wp.rearrange("(i p) n -> p i n", p=P))

        # warm up the PE clock-gate (HAM) while input DMAs are in flight
        warm = persist.tile([P, QB], BF16, tag="warm", name="warm")
        nc.vector.memset(warm[:], 0.0)
        wps = ps_u.tile([P, QB], F32, tag="u", name="uwarm")
        for _ in range(9):
            nc.tensor.matmul(wps[:], warm[:, 0:P], warm[:], start=True, stop=True)

        def xsrc(c, lo, hi):
            if hi <= QB:
                return x0_sb[:][:, c, lo:hi]
            if hi <= TH:
                return xh_sb[c][0][:, lo:hi]
            return xh_sb[c][1][:, lo - TH : hi - TH]

        def wview(hp, t, c):
            return wqk_sb[:][:, hp, c, t, :]

        sums_d = dram.tile([n_hp * n_qb, 2 * QB], F32, tag="sums_d", name="sums_d")
        recips_d = dram.tile(
            [n_hp * n_qb, 2 * QB], F32, tag="recips_d", name="recips_d"
        )

        # ---- v: interleaved ones column per head: v1 [T, HL*(D+1)] ----
        v1_sb = [
            persist.tile([P, HL * D1], BF16, tag=f"v1_{tt}", name=f"v1_{tt}")
            for tt in range(n_tt)
        ]

        def emit_v(tts):
            for tt in tts:
                t = v1_sb[tt]
                ones_view = t[:].rearrange("p (h e) -> p h e", h=HL)[:, :, 0:1]
                nc.vector.memset(ones_view, 1.0)
                ps = ps_u.tile([P, CL], F32, tag="u", name="uv")
                for c in range(n_ct):
                    nc.tensor.matmul(
                        ps[:],
                        xsrc(c, tt * P, (tt + 1) * P),
                        wv_sb[c][:],
                        start=(c == 0),
                        stop=(c == n_ct - 1),
                    )
                dst_view = t[:].rearrange("p (h e) -> p h e", h=HL)[:, :, 1 : D + 1]
                src_view = ps[:].rearrange("p (h e) -> p h e", h=HL)
                nc.vector.tensor_copy(dst_view, src_view)

        # ---- q/k projections (d-major) ----
        yT_sb = [
            persist.tile([P, T], BF16, tag=f"yT{m}", name=f"yT{m}")
            for m in range(n_mt)
        ]
        q_d = [[] for _ in range(n_mt)]
        k_d = [[] for _ in range(n_mt)]

        def emit_qk(hp, bs):
            # ascending contiguous b's per hp
            for b in bs:
                for t, name, dst in ((0, "k", k_d), (1, "q", q_d)):
                    tl = persist.tile(
                        [P, QB], BF16, tag=f"{name}d{hp}_{b}", name=f"{name}d{hp}_{b}"
                    )
                    dst[hp].append(tl)
                    ps = ps_u.tile([P, QB], F32, tag="u", name="uqk")
                    for c in range(n_ct):
                        nc.tensor.matmul(
                            ps[:],
                            wview(hp, t, c),
                            xsrc(c, b * QB, (b + 1) * QB),
                            start=(c == 0),
                            stop=(c == n_ct - 1),
                        )
                    nc.vector.tensor_copy(tl[:], ps[:])

        def emit_attn(hp, qb):
            yts = ps_av.tile([D1, 2 * QB], F32, tag="av", name="yts")
            n_kt = dpb * qb + dpb

            def emit_av(kt, pt):
                q0 = P * max(kt - dpb * qb, 0)
                for i in range(2):
                    h = 2 * hp + i
                    nc.tensor.matmul(
                        yts[:, i * QB + q0 : (i + 1) * QB],
                        v1_sb[kt][:, h * D1 : (h + 1) * D1],
                        pt[:, i * QB + q0 : (i + 1) * QB],
                        start=(kt == 0),
                        stop=(kt == n_kt - 1),
                        skip_group_check=True,
                    )

            pending = []
            for kt in range(n_kt):
                m = kt - dpb * qb  # >=0: diagonal tile index
                s0 = P * max(m, 0)
                st = ps_sc.tile([P, 2 * QB], F32, tag="sc", name="st")
                for i in range(2):
                    base = 64 * i
                    nc.tensor.matmul(
                        st[:, i * QB + s0 : (i + 1) * QB],
                        k_d[hp][kt // dpb][
                            base : base + 64, (kt % dpb) * P : (kt % dpb + 1) * P
                        ],
                        q_d[hp][qb][base : base + 64, s0:],
                        start=True,
                        stop=True,
                    )
                pt = ppool.tile([P, 2 * QB], BF16, tag="pt", name="pt")
                if m <= 0:
                    nc.scalar.activation(
                        pt[:], st[:], mybir.ActivationFunctionType.Exp, scale=scale
                    )
                else:
                    nc.scalar.activation(
                        pt[:].rearrange("p (i q) -> p i q", i=2)[:, :, s0:],
                        st[:].rearrange("p (i q) -> p i q", i=2)[:, :, s0:],
                        mybir.ActivationFunctionType.Exp,
                        scale=scale,
                    )
                if m >= 0:
                    q0 = P * m
                    sl = pt[:].rearrange("p (i q) -> p i q", i=2)[:, :, q0 : q0 + P]
                    nc.gpsimd.tensor_mul(
                        sl, sl, trimask[:, None, :].broadcast_to([P, 2, P])
                    )
                pending.append((kt, pt))
                if len(pending) > 2:
                    emit_av(*pending.pop(0))
            for item in pending:
                emit_av(*item)

            # epilogue: one PSUM->SBUF copy for both heads; write unnormalized
            # y into yT (casting SWDGE, both head halves in one DMA)
            ys = spool.tile([D1, 2 * QB], F32, tag="ys", name="ys")
            nc.vector.tensor_copy(ys[:], yts[:])
            for i in range(2):
                nc.gpsimd.dma_start(
                    yT_sb[hp][64 * i : 64 * i + 64, qb * QB : (qb + 1) * QB],
                    ys[1 : D + 1, i * QB : (i + 1) * QB],
                )
            if hp < n_hp - 1 or qb < n_qb - 1:
                nc.sync.dma_start(
                    sums_d[hp * n_qb + qb : hp * n_qb + qb + 1, :], ys[0:1, :]
                )
            return ys

        bc_tiles = {}  # (hp, qb) -> broadcast reciprocal tile

        def emit_bc(hp, qb):
            r = hp * n_qb + qb
            bc = bcpool.tile([P, QB], F32, tag="bc", name="bc")
            for i in range(2):
                nc.sync.dma_start(
                    bc[64 * i : 64 * i + 64, :],
                    recips_d[r : r + 1, i * QB : (i + 1) * QB].to_broadcast(
                        (64, QB)
                    ),
                )
            bc_tiles[(hp, qb)] = bc

        def emit_norm_chain(hp):
            # hp0-2: batched at head-pair end; latency hidden by next hp
            r0 = hp * n_qb
            allsums = stage.tile([n_qb, 2 * QB], F32, tag="st_s", name="allsums")
            nc.sync.dma_start(allsums[:], sums_d[r0 : r0 + n_qb, :])
            allrec = stage.tile([n_qb, 2 * QB], F32, tag="st_s", name="allrec")
            nc.vector.reciprocal_approx_fast(allrec[:], allsums[:])
            nc.sync.dma_start(recips_d[r0 : r0 + n_qb, :], allrec[:])
            for qb in range(n_qb):
                emit_bc(hp, qb)

        ones64 = persist.tile([1, 64], F32, tag="ones64", name="ones64")
        nc.vector.memset(ones64[:], 1.0)

        def emit_norm3_chain(qb, ys):
            # hp3: per-q-block. reciprocal_approx_fast misbehaves on a
            # 1-partition AP on HW, so first broadcast the ys sum row across
            # partitions. The last q-block is latency-critical (serial tail):
            # use a PE K=1 broadcast matmul + 128-partition reciprocal with
            # no DMA hops; earlier blocks take the (cheaper, hidden) DRAM
            # gather path.
            hp = n_hp - 1
            r = hp * n_qb + qb
            if qb == n_qb - 1:
                sb = ps_u.tile([P, QB], F32, tag="u", name="ubc")
                for i in range(2):
                    nc.tensor.matmul(
                        sb[64 * i : 64 * i + 64, :],
                        ones64[:],
                        ys[0:1, i * QB : (i + 1) * QB],
                        start=True,
                        stop=True,
                    )
                bc = bcpool.tile([P, QB], F32, tag="bc", name="bc")
                nc.vector.reciprocal_approx_fast(bc[:], sb[:])
                bc_tiles[(hp, qb)] = bc
            else:
                two = stage.tile([2, QB], F32, tag="st_r", name="sums2")
                nc.sync.dma_start(
                    two[:], sums_d[r : r + 1, :].rearrange("o (p c) -> (o p) c", p=2)
                )
                rec2 = stage.tile([2, QB], F32, tag="st_r", name="rec2")
                nc.vector.reciprocal_approx_fast(rec2[:], two[:])
                nc.sync.dma_start(
                    recips_d[r : r + 1, :].rearrange("o (p c) -> (o p) c", p=2),
                    rec2[:],
                )
                emit_bc(hp, qb)

        def emit_norm_muls(hp):
            for qb in range(n_qb):
                sl = yT_sb[hp][:, qb * QB : (qb + 1) * QB]
                nc.vector.tensor_mul(sl, sl, bc_tiles[(hp, qb)][:])

        def emit_proj(qb):
            # normalize hp3's block first, then project t-range of this qb
            hp = n_hp - 1
            sl = yT_sb[hp][:, qb * QB : (qb + 1) * QB]
            nc.vector.tensor_mul(sl, sl, bc_tiles[(hp, qb)][:])
            for tt in range(dpb * qb, dpb * (qb + 1)):
                ost = opool.tile([P, n_cb * QB], BF16, tag="ost", name="ost")
                for cb in range(n_cb):
                    ps = ps_u.tile([P, QB], F32, tag="u", name="upj")
                    for m in range(n_mt):
                        nc.tensor.matmul(
                            ps[:],
                            yT_sb[m][:, tt * P : (tt + 1) * P],
                            wp_sb[:][:, m, cb * QB : (cb + 1) * QB],
                            start=(m == 0),
                            stop=(m == n_mt - 1),
                        )
                    nc.vector.tensor_copy(ost[:, cb * QB : (cb + 1) * QB], ps[:])
                nc.sync.dma_start(out[tt * P : (tt + 1) * P, :], ost[:])

        # ---- schedule ----
        emit_qk(0, [0])
        emit_v(range(0, dpb))
        for qb in range(n_qb):
            emit_attn(0, qb)
            if qb < n_qb - 1:
                emit_qk(0, [qb + 1])
                emit_v(range(dpb * (qb + 1), dpb * (qb + 2)))
            emit_qk(1, [qb])
        emit_norm_chain(0)
        for hp in range(1, n_hp):
            for qb in range(n_qb):
                ys_last = emit_attn(hp, qb)
                if hp < n_hp - 2:
                    emit_qk(hp + 1, [qb])
                elif hp == n_hp - 2:
                    if qb < n_qb - 1:
                        emit_qk(hp + 1, [qb])
                else:
                    if qb == 0:
                        emit_qk(hp, [n_qb - 1])
                    emit_norm3_chain(qb, ys_last)
                    if qb > 0:
                        emit_proj(qb - 1)
                if qb == 0:
                    emit_norm_muls(hp - 1)
            if hp < n_hp - 1:
                emit_norm_chain(hp)
        emit_proj(n_qb - 1)

    nc.compile()
    return nc


_PROGRAM_CACHE = {}


def _get_program(C_eff):
    key = C_eff
    if key not in _PROGRAM_CACHE:
        _PROGRAM_CACHE[key] = build_kernel(T=T, C=C_eff, HL=HL, D=D, Cout=C)
    return _PROGRAM_CACHE[key]


def _make_in_maps(x, Wqkv, bqkv):
    """Shard + cast inputs for the 8 cores. Returns (in_maps, C_eff)."""
    if np.any(bqkv):
        # Fold the qkv bias in as an extra contraction row (x gains a ones
        # column), zero-padded up to a multiple of 128.
        C_eff = ((C + 1 + P - 1) // P) * P
        Waug = np.zeros((C_eff, 3 * C), dtype=np.float32)
        Waug[:C] = Wqkv
        Waug[C] = bqkv
    else:
        C_eff = C
        Waug = Wqkv

    n_hp = HL // 2
    masks = (np.arange(P)[:, None] <= np.arange(P)[None, :]).astype(NPBF16)
    in_maps = []
    for core in range(N_CORES):
        b, hh = divmod(core, N_CORES // B)
        xT = np.zeros((C_eff, T), dtype=np.float32)
        xT[:C] = x[b].T
        if C_eff > C:
            xT[C] = 1.0
        c0 = hh * CL
        # wqk packed hp-major with k|q side by side: [hp, C_eff, 2*128],
        # then half-packed into wide rows: row r = [chunk[r] | chunk[C/2+r]]
        wqk = np.empty((n_hp, C_eff, 2 * P), dtype=np.float32)
        for hp in range(n_hp):
            wqk[hp, :, :P] = Waug[:, 1 * C + c0 + hp * P : 1 * C + c0 + (hp + 1) * P]
            wqk[hp, :, P:] = Waug[:, 0 * C + c0 + hp * P : 0 * C + c0 + (hp + 1) * P]
        wqkp = np.concatenate([wqk[:, : C_eff // 2], wqk[:, C_eff // 2 :]], axis=2)
        xTb = xT.astype(NPBF16)
        x0p = np.concatenate(
            [xTb[: C_eff // 2, :QB], xTb[C_eff // 2 :, :QB]], axis=1
        )
        in_maps.append(
            {
                "x0T": np.ascontiguousarray(x0p),
                "xT": xTb,
                "wqk": np.ascontiguousarray(wqkp).astype(NPBF16),
                "wv": np.ascontiguousarray(
                    Waug[:, 2 * C + c0 : 2 * C + c0 + CL]
                ).astype(NPBF16),
                "wp": None,  # filled below (depends only on hh)
                "masks": masks,
            }
        )
    return in_maps, C_eff


def _run(x, Wqkv, bqkv, Wproj, bproj, trace=False):
    from concourse.bass_utils import run_bass_kernel_spmd

    in_maps, C_eff = _make_in_maps(x, Wqkv, bqkv)
    wp_by_hh = [
        np.ascontiguousarray(Wproj[hh * CL : (hh + 1) * CL, :]).astype(NPBF16)
        for hh in range(N_CORES // B)
    ]
    for core in range(N_CORES):
        in_maps[core]["wp"] = wp_by_hh[core % (N_CORES // B)]

    nc = _get_program(C_eff)
    res = run_bass_kernel_spmd(
        nc, in_maps, core_ids=list(range(N_CORES)), trace=trace
    )

    halves = N_CORES // B
    y = np.empty((B, T, C), dtype=np.float32)
    for b in range(B):
        acc = res.results[b * halves]["out"].astype(np.float32)
        for hh in range(1, halves):
            acc = acc + res.results[b * halves + hh]["out"].astype(np.float32)
        y[b] = acc + bproj.astype(np.float32)
    return y, res


def kernel(x, Wqkv, bqkv, Wproj, bproj):
    y, _ = _run(
        np.asarray(x, dtype=np.float32),
        np.asarray(Wqkv, dtype=np.float32),
        np.asarray(bqkv, dtype=np.float32),
        np.asarray(Wproj, dtype=np.float32),
        np.asarray(bproj, dtype=np.float32),
        trace=False,
    )
    return y


if __name__ == "__main__":
    # quick compile check
    build_kernel()
    print("compile OK")


# revision 17
# speedup vs baseline: 1.0152x; 1.0152x over previous
"""Causal self-attention on 8 TRN2 NeuronCores (Bass/Tile, SPMD).

Problem: y = CausalSelfAttention(x; Wqkv, bqkv, Wproj, bproj)
  x [B=4, T=2048, C=1024], H=16 heads, D=64.

Sharding: core c = (batch b = c//2, head-half hh = c%2). Each core computes
q/k/v for its 8 heads of its batch (Wqkv column-sharded), full causal
attention for those heads, and a partial output projection (Wproj
row-sharded). Host sums the two bf16 partials per batch and adds bproj.

Schedule (v2): the PE is the bottleneck (~225us of streamed columns), so
every phase interleaves "fill" matmuls into the ACT-paced attention loop:
  - hp0 attention is interleaved with v-projection tiles and the next
    q-block's q/k projection; hp1/hp2 with the next head-pair's q/k.
  - hp3 attention is interleaved with the *output projection* of the
    previous q-block (its softmax denominators are normalized per-q-block
    instead of per-head-pair), so only the last q-block's projection
    remains as a serial tail.
  - Input DMAs are spread across the sync/scalar/vector queues, with a
    duplicated first-q-block slice of x (x0) so the first q/k projection
    starts before the full x transfer lands.
  - Both heads' AV accumulate into one [65, 1024] PSUM tile, halving the
    epilogue copies/DMAs; causal mask multiplies run on GpSimd.
"""

import math
from contextlib import ExitStack

import numpy as np
import ml_dtypes

import concourse.tile as tile
from concourse import bacc, mybir

BF16 = mybir.dt.bfloat16
F32 = mybir.dt.float32
NPBF16 = ml_dtypes.bfloat16

P = 128  # partitions / k-tile size
QB = 512  # q-block (matmul N; one fp32 PSUM bank)

B, T, C, H, D = 4, 2048, 1024, 16, 64
N_CORES = 8
HL = H // (N_CORES // B)  # heads per core (8)
CL = HL * D  # local head width (512)

# ---------------------------------------------------------------------------
# Per-core Bass program
# ---------------------------------------------------------------------------


def build_kernel(T=T, C=C, HL=HL, D=D, Cout=C):
    CL = HL * D
    n_ct = C // P
    n_mt = CL // P
    n_tt = T // P
    n_qb = T // QB
    n_hp = HL // 2
    dpb = QB // P
    n_cb = Cout // QB
    scale = 1.0 / math.sqrt(D)
    D1 = D + 1
    ca = (n_ct + 1) // 2  # x c-tiles in chunk a (rest in chunk b)
    cb_n = n_ct - ca

    assert C % P == 0 and CL % P == 0 and T % QB == 0 and Cout % QB == 0
    assert HL % 2 == 0 and D == 64 and n_mt == n_hp

    nc = bacc.Bacc("TRN2", target_bir_lowering=False, debug=False)
    x0T = nc.dram_tensor("x0T", [C, QB], BF16, kind="ExternalInput")
    xT = nc.dram_tensor("xT", [C, T], BF16, kind="ExternalInput")
    # per-hp [C, 2*P] chunks, k|q side by side (wide DMA rows)
    wqk = nc.dram_tensor("wqk", [n_hp, C, 2 * P], BF16, kind="ExternalInput")
    wv = nc.dram_tensor("wv", [C, CL], BF16, kind="ExternalInput")
    wp = nc.dram_tensor("wp", [CL, Cout], BF16, kind="ExternalInput")
    masks = nc.dram_tensor("masks", [P, P], BF16, kind="ExternalInput")
    out = nc.dram_tensor("out", [T, Cout], BF16, kind="ExternalOutput")

    with tile.TileContext(nc) as tc, ExitStack() as ctx:
        persist = ctx.enter_context(tc.tile_pool(name="persist", bufs=1))
        # PSUM budget (8 banks): scores 2x[128,1024] + AV 1x[65,1024] + u 2x[128,512]
        ps_sc = ctx.enter_context(tc.tile_pool(name="ps_sc", bufs=2, space="PSUM"))
        ps_av = ctx.enter_context(tc.tile_pool(name="ps_av", bufs=1, space="PSUM"))
        ps_u = ctx.enter_context(tc.tile_pool(name="ps_u", bufs=2, space="PSUM"))
        ppool = ctx.enter_context(tc.tile_pool(name="ppool", bufs=6))
        spool = ctx.enter_context(tc.tile_pool(name="spool", bufs=3))
        bcpool = ctx.enter_context(tc.tile_pool(name="bcpool", bufs=6))
        stage = ctx.enter_context(tc.tile_pool(name="stage", bufs=3))
        opool = ctx.enter_context(tc.tile_pool(name="opool", bufs=2))
        dram = ctx.enter_context(tc.tile_pool(name="dram", bufs=1, space="DRAM"))

        # ---- persistent input tiles; fine-grained DMAs for pipelining,
        # spread over the scalar and sync queues by first-use time ----
        x0_sb = persist.tile([P, n_ct, QB], BF16, tag="x0", name="x0")
        TH = T // 2
        xh_sb = [
            [
                persist.tile([P, TH], BF16, tag=f"x{c}_{h}", name=f"x{c}_{h}")
                for h in range(2)
            ]
            for c in range(n_ct)
        ]
        wqk_sb = persist.tile([P, n_hp, n_ct, 2, P], BF16, tag="wqk", name="wqk")
        wv_sb = [
            persist.tile([P, CL], BF16, tag=f"wv{c}", name=f"wv{c}")
            for c in range(n_ct)
        ]
        wp_sb = persist.tile([P, n_mt, Cout], BF16, tag="wp", name="wp")
        trimask = persist.tile([P, P], BF16, tag="trimask", name="trimask")

        def dma_wqk(hp):
            nc.sync.dma_start(
                wqk_sb[:][:, hp, :, :, :],
                wqk[hp].rearrange("(i p) (t w) -> p i t w", p=P, w=P),
            )

        # scalar queue: x0 slice first (unblocks qk0+v0), then x first halves
        nc.scalar.dma_start(x0_sb[:], x0T.rearrange("(i p) c -> p i c", p=P))
        for c in range(n_ct):
            nc.scalar.dma_start(xh_sb[c][0][:], xT[c * P : (c + 1) * P, 0:TH])
        # sync queue: hp0 q/k weights, wv tiles, x second halves, the rest
        dma_wqk(0)
        for c in range(n_ct):
            nc.sync.dma_start(wv_sb[c][:], wv[c * P : (c + 1) * P, :])
        for c in range(n_ct):
            nc.sync.dma_start(xh_sb[c][1][:], xT[c * P : (c + 1) * P, TH:])
        nc.sync.dma_start(trimask[:], masks[:])
        for hp in range(1, n_hp):
            dma_wqk(hp)
        nc.sync.dma_start(wp_sb[:], wp.rearrange("(i p) n -> p i n", p=P))

        # warm up the PE clock-gate (HAM) while input DMAs are in flight
        warm = persist.tile([P, QB], BF16, tag="warm", name="warm")
        nc.vector.memset(warm[:], 0.0)
        wps = ps_u.tile([P, QB], F32, tag="u", name="uwarm")
        for _ in range(14):
            nc.tensor.matmul(wps[:], warm[:, 0:P], warm[:], start=True, stop=True)

        def xsrc(c, lo, hi):
            if hi <= QB:
                return x0_sb[:][:, c, lo:hi]
            if hi <= TH:
                return xh_sb[c][0][:, lo:hi]
            return xh_sb[c][1][:, lo - TH : hi - TH]

        def wview(hp, t, c):
            return wqk_sb[:][:, hp, c, t, :]

        sums_d = dram.tile([n_hp * n_qb, 2 * QB], F32, tag="sums_d", name="sums_d")
        recips_d = dram.tile(
            [n_hp * n_qb, 2 * QB], F32, tag="recips_d", name="recips_d"
        )

        # ---- v: interleaved ones column per head: v1 [T, HL*(D+1)] ----
        v1_sb = [
            persist.tile([P, HL * D1], BF16, tag=f"v1_{tt}", name=f"v1_{tt}")
            for tt in range(n_tt)
        ]

        def emit_v(tts):
            for tt in tts:
                t = v1_sb[tt]
                ones_view = t[:].rearrange("p (h e) -> p h e", h=HL)[:, :, 0:1]
                nc.vector.memset(ones_view, 1.0)
                ps = ps_u.tile([P, CL], F32, tag="u", name="uv")
                for c in range(n_ct):
                    nc.tensor.matmul(
                        ps[:],
                        xsrc(c, tt * P, (tt + 1) * P),
                        wv_sb[c][:],
                        start=(c == 0),
                        stop=(c == n_ct - 1),
                    )
                dst_view = t[:].rearrange("p (h e) -> p h e", h=HL)[:, :, 1 : D + 1]
                src_view = ps[:].rearrange("p (h e) -> p h e", h=HL)
                nc.vector.tensor_copy(dst_view, src_view)

        # ---- q/k projections (d-major) ----
        yT_sb = [
            persist.tile([P, T], BF16, tag=f"yT{m}", name=f"yT{m}")
            for m in range(n_mt)
        ]
        q_d = [[] for _ in range(n_mt)]
        k_d = [[] for _ in range(n_mt)]

        def emit_qk(hp, bs):
            # ascending contiguous b's per hp
            for b in bs:
                for t, name, dst in ((0, "k", k_d), (1, "q", q_d)):
                    tl = persist.tile(
                        [P, QB], BF16, tag=f"{name}d{hp}_{b}", name=f"{name}d{hp}_{b}"
                    )
                    dst[hp].append(tl)
                    ps = ps_u.tile([P, QB], F32, tag="u", name="uqk")
                    for c in range(n_ct):
                        nc.tensor.matmul(
                            ps[:],
                            wview(hp, t, c),
                            xsrc(c, b * QB, (b + 1) * QB),
                            start=(c == 0),
                            stop=(c == n_ct - 1),
                        )
                    nc.vector.tensor_copy(tl[:], ps[:])

        def emit_attn(hp, qb):
            yts = ps_av.tile([D1, 2 * QB], F32, tag="av", name="yts")
            n_kt = dpb * qb + dpb

            def emit_av(kt, pt):
                q0 = P * max(kt - dpb * qb, 0)
                for i in range(2):
                    h = 2 * hp + i
                    nc.tensor.matmul(
                        yts[:, i * QB + q0 : (i + 1) * QB],
                        v1_sb[kt][:, h * D1 : (h + 1) * D1],
                        pt[:, i * QB + q0 : (i + 1) * QB],
                        start=(kt == 0),
                        stop=(kt == n_kt - 1),
                        skip_group_check=True,
                    )

            pending = []
            for kt in range(n_kt):
                m = kt - dpb * qb  # >=0: diagonal tile index
                s0 = P * max(m, 0)
                st = ps_sc.tile([P, 2 * QB], F32, tag="sc", name="st")
                for i in range(2):
                    base = 64 * i
                    nc.tensor.matmul(
                        st[:, i * QB + s0 : (i + 1) * QB],
                        k_d[hp][kt // dpb][
                            base : base + 64, (kt % dpb) * P : (kt % dpb + 1) * P
                        ],
                        q_d[hp][qb][base : base + 64, s0:],
                        start=True,
                        stop=True,
                    )
                pt = ppool.tile([P, 2 * QB], BF16, tag="pt", name="pt")
                if m <= 0:
                    nc.scalar.activation(
                        pt[:], st[:], mybir.ActivationFunctionType.Exp, scale=scale
                    )
                else:
                    nc.scalar.activation(
                        pt[:].rearrange("p (i q) -> p i q", i=2)[:, :, s0:],
                        st[:].rearrange("p (i q) -> p i q", i=2)[:, :, s0:],
                        mybir.ActivationFunctionType.Exp,
                        scale=scale,
                    )
                if m >= 0:
                    q0 = P * m
                    sl = pt[:].rearrange("p (i q) -> p i q", i=2)[:, :, q0 : q0 + P]
                    nc.gpsimd.tensor_mul(
                        sl, sl, trimask[:, None, :].broadcast_to([P, 2, P])
                    )
                pending.append((kt, pt))
                if len(pending) > 2:
                    emit_av(*pending.pop(0))
            for item in pending:
                emit_av(*item)

            # epilogue: one PSUM->SBUF copy for both heads; write unnormalized
            # y into yT (casting SWDGE, both head halves in one DMA)
            ys = spool.tile([D1, 2 * QB], F32, tag="ys", name="ys")
            nc.vector.tensor_copy(ys[:], yts[:])
            for i in range(2):
                nc.gpsimd.dma_start(
                    yT_sb[hp][64 * i : 64 * i + 64, qb * QB : (qb + 1) * QB],
                    ys[1 : D + 1, i * QB : (i + 1) * QB],
                )
            if hp < n_hp - 1 or qb < n_qb - 1:
                nc.sync.dma_start(
                    sums_d[hp * n_qb + qb : hp * n_qb + qb + 1, :], ys[0:1, :]
                )
            return ys

        bc_tiles = {}  # (hp, qb) -> broadcast reciprocal tile

        def emit_bc(hp, qb):
            r = hp * n_qb + qb
            bc = bcpool.tile([P, QB], F32, tag="bc", name="bc")
            for i in range(2):
                nc.sync.dma_start(
                    bc[64 * i : 64 * i + 64, :],
                    recips_d[r : r + 1, i * QB : (i + 1) * QB].to_broadcast(
                        (64, QB)
                    ),
                )
            bc_tiles[(hp, qb)] = bc

        def emit_norm_chain(hp):
            # hp0-2: batched at head-pair end; latency hidden by next hp
            r0 = hp * n_qb
            allsums = stage.tile([n_qb, 2 * QB], F32, tag="st_s", name="allsums")
            nc.sync.dma_start(allsums[:], sums_d[r0 : r0 + n_qb, :])
            allrec = stage.tile([n_qb, 2 * QB], F32, tag="st_s", name="allrec")
            nc.vector.reciprocal_approx_fast(allrec[:], allsums[:])
            nc.sync.dma_start(recips_d[r0 : r0 + n_qb, :], allrec[:])
            for qb in range(n_qb):
                emit_bc(hp, qb)

        ones64 = persist.tile([1, 64], F32, tag="ones64", name="ones64")
        nc.vector.memset(ones64[:], 1.0)

        def emit_norm3_chain(qb, ys):
            # hp3: per-q-block. reciprocal_approx_fast misbehaves on a
            # 1-partition AP on HW, so first broadcast the ys sum row across
            # partitions. The last q-block is latency-critical (serial tail):
            # use a PE K=1 broadcast matmul + 128-partition reciprocal with
            # no DMA hops; earlier blocks take the (cheaper, hidden) DRAM
            # gather path.
            hp = n_hp - 1
            r = hp * n_qb + qb
            if qb == n_qb - 1:
                sb = ps_u.tile([P, QB], F32, tag="u", name="ubc")
                for i in range(2):
                    nc.tensor.matmul(
                        sb[64 * i : 64 * i + 64, :],
                        ones64[:],
                        ys[0:1, i * QB : (i + 1) * QB],
                        start=True,
                        stop=True,
                    )
                bc = bcpool.tile([P, QB], F32, tag="bc", name="bc")
                nc.vector.reciprocal_approx_fast(bc[:], sb[:])
                bc_tiles[(hp, qb)] = bc
            else:
                two = stage.tile([2, QB], F32, tag="st_r", name="sums2")
                nc.sync.dma_start(
                    two[:], sums_d[r : r + 1, :].rearrange("o (p c) -> (o p) c", p=2)
                )
                rec2 = stage.tile([2, QB], F32, tag="st_r", name="rec2")
                nc.vector.reciprocal_approx_fast(rec2[:], two[:])
                nc.sync.dma_start(
                    recips_d[r : r + 1, :].rearrange("o (p c) -> (o p) c", p=2),
                    rec2[:],
                )
                emit_bc(hp, qb)

        def emit_norm_muls(hp):
            for qb in range(n_qb):
                sl = yT_sb[hp][:, qb * QB : (qb + 1) * QB]
                nc.vector.tensor_mul(sl, sl, bc_tiles[(hp, qb)][:])

        def emit_proj(qb):
            # normalize hp3's block first, then project t-range of this qb
            hp = n_hp - 1
            sl = yT_sb[hp][:, qb * QB : (qb + 1) * QB]
            nc.vector.tensor_mul(sl, sl, bc_tiles[(hp, qb)][:])
            for tt in range(dpb * qb, dpb * (qb + 1)):
                ost = opool.tile([P, n_cb * QB], BF16, tag="ost", name="ost")
                for cb in range(n_cb):
                    ps = ps_u.tile([P, QB], F32, tag="u", name="upj")
                    for m in range(n_mt):
                        nc.tensor.matmul(
                            ps[:],
                            yT_sb[m][:, tt * P : (tt + 1) * P],
                            wp_sb[:][:, m, cb * QB : (cb + 1) * QB],
                            start=(m == 0),
                            stop=(m == n_mt - 1),
                        )
                    nc.vector.tensor_copy(ost[:, cb * QB : (cb + 1) * QB], ps[:])
                nc.sync.dma_start(out[tt * P : (tt + 1) * P, :], ost[:])

        # ---- schedule ----
        emit_qk(0, [0])
        emit_v(range(0, dpb))
        for qb in range(n_qb):
            emit_attn(0, qb)
            if qb < n_qb - 1:
                emit_qk(0, [qb + 1])
                emit_v(range(dpb * (qb + 1), dpb * (qb + 2)))
            emit_qk(1, [qb])
        emit_norm_chain(0)
        for hp in range(1, n_hp):
            for qb in range(n_qb):
                ys_last = emit_attn(hp, qb)
                if hp < n_hp - 2:
                    emit_qk(hp + 1, [qb])
                elif hp == n_hp - 2:
                    if qb < n_qb - 1:
                        emit_qk(hp + 1, [qb])
                else:
                    if qb == 0:
                        emit_qk(hp, [n_qb - 1])
                    emit_norm3_chain(qb, ys_last)
                    if qb > 0:
                        emit_proj(qb - 1)
                if qb == 0:
                    emit_norm_muls(hp - 1)
            if hp < n_hp - 1:
                emit_norm_chain(hp)
        emit_proj(n_qb - 1)

    nc.compile()
    return nc


_PROGRAM_CACHE = {}


def _get_program(C_eff):
    key = C_eff
    if key not in _PROGRAM_CACHE:
        _PROGRAM_CACHE[key] = build_kernel(T=T, C=C_eff, HL=HL, D=D, Cout=C)
    return _PROGRAM_CACHE[key]


def _make_in_maps(x, Wqkv, bqkv):
    """Shard + cast inputs for the 8 cores. Returns (in_maps, C_eff)."""
    if np.any(bqkv):
        # Fold the qkv bias in as an extra contraction row (x gains a ones
        # column), zero-padded up to a multiple of 128.
        C_eff = ((C + 1 + P - 1) // P) * P
        Waug = np.zeros((C_eff, 3 * C), dtype=np.float32)
        Waug[:C] = Wqkv
        Waug[C] = bqkv
    else:
        C_eff = C
        Waug = Wqkv

    n_hp = HL // 2
    masks = (np.arange(P)[:, None] <= np.arange(P)[None, :]).astype(NPBF16)
    in_maps = []
    for core in range(N_CORES):
        b, hh = divmod(core, N_CORES // B)
        xT = np.zeros((C_eff, T), dtype=np.float32)
        xT[:C] = x[b].T
        if C_eff > C:
            xT[C] = 1.0
        c0 = hh * CL
        # wqk packed hp-major with k|q side by side: [hp, C_eff, 2*128]
        wqk = np.empty((n_hp, C_eff, 2 * P), dtype=np.float32)
        for hp in range(n_hp):
            wqk[hp, :, :P] = Waug[:, 1 * C + c0 + hp * P : 1 * C + c0 + (hp + 1) * P]
            wqk[hp, :, P:] = Waug[:, 0 * C + c0 + hp * P : 0 * C + c0 + (hp + 1) * P]
        xTb = xT.astype(NPBF16)
        in_maps.append(
            {
                "x0T": np.ascontiguousarray(xTb[:, :QB]),
                "xT": xTb,
                "wqk": wqk.astype(NPBF16),
                "wv": np.ascontiguousarray(
                    Waug[:, 2 * C + c0 : 2 * C + c0 + CL]
                ).astype(NPBF16),
                "wp": None,  # filled below (depends only on hh)
                "masks": masks,
            }
        )
    return in_maps, C_eff


def _run(x, Wqkv, bqkv, Wproj, bproj, trace=False):
    from concourse.bass_utils import run_bass_kernel_spmd

    in_maps, C_eff = _make_in_maps(x, Wqkv, bqkv)
    wp_by_hh = [
        np.ascontiguousarray(Wproj[hh * CL : (hh + 1) * CL, :]).astype(NPBF16)
        for hh in range(N_CORES // B)
    ]
    for core in range(N_CORES):
        in_maps[core]["wp"] = wp_by_hh[core % (N_CORES // B)]

    nc = _get_program(C_eff)
    res = run_bass_kernel_spmd(
        nc, in_maps, core_ids=list(range(N_CORES)), trace=trace
    )

    halves = N_CORES // B
    y = np.empty((B, T, C), dtype=np.float32)
    for b in range(B):
        acc = res.results[b * halves]["out"].astype(np.float32)
        for hh in range(1, halves):
            acc = acc + res.results[b * halves + hh]["out"].astype(np.float32)
        y[b] = acc + bproj.astype(np.float32)
    return y, res


def kernel(x, Wqkv, bqkv, Wproj, bproj):
    y, _ = _run(
        np.asarray(x, dtype=np.float32),
        np.asarray(Wqkv, dtype=np.float32),
        np.asarray(bqkv, dtype=np.float32),
        np.asarray(Wproj, dtype=np.float32),
        np.asarray(bproj, dtype=np.float32),
        trace=False,
    )
    return y


if __name__ == "__main__":
    # quick compile check
    build_kernel()
    print("compile OK")


# revision 18
# speedup vs baseline: 1.0299x; 1.0146x over previous
"""Causal self-attention on 8 TRN2 NeuronCores (Bass/Tile, SPMD).

Problem: y = CausalSelfAttention(x; Wqkv, bqkv, Wproj, bproj)
  x [B=4, T=2048, C=1024], H=16 heads, D=64.

Sharding: core c = (batch b = c//2, head-half hh = c%2). Each core computes
q/k/v for its 8 heads of its batch (Wqkv column-sharded), full causal
attention for those heads, and a partial output projection (Wproj
row-sharded). Host sums the two bf16 partials per batch and adds bproj.

Schedule (v2): the PE is the bottleneck (~225us of streamed columns), so
every phase interleaves "fill" matmuls into the ACT-paced attention loop:
  - hp0 attention is interleaved with v-projection tiles and the next
    q-block's q/k projection; hp1/hp2 with the next head-pair's q/k.
  - hp3 attention is interleaved with the *output projection* of the
    previous q-block (its softmax denominators are normalized per-q-block
    instead of per-head-pair), so only the last q-block's projection
    remains as a serial tail.
  - Input DMAs are spread across the sync/scalar/vector queues, with a
    duplicated first-q-block slice of x (x0) so the first q/k projection
    starts before the full x transfer lands.
  - Both heads' AV accumulate into one [65, 1024] PSUM tile, halving the
    epilogue copies/DMAs; causal mask multiplies run on GpSimd.
"""

import math
from contextlib import ExitStack

import numpy as np
import ml_dtypes

import concourse.tile as tile
from concourse import bacc, mybir

BF16 = mybir.dt.bfloat16
F32 = mybir.dt.float32
NPBF16 = ml_dtypes.bfloat16

P = 128  # partitions / k-tile size
QB = 512  # q-block (matmul N; one fp32 PSUM bank)

B, T, C, H, D = 4, 2048, 1024, 16, 64
N_CORES = 8
HL = H // (N_CORES // B)  # heads per core (8)
CL = HL * D  # local head width (512)

# ---------------------------------------------------------------------------
# Per-core Bass program
# ---------------------------------------------------------------------------


def build_kernel(T=T, C=C, HL=HL, D=D, Cout=C):
    CL = HL * D
    n_ct = C // P
    n_mt = CL // P
    n_tt = T // P
    n_qb = T // QB
    n_hp = HL // 2
    dpb = QB // P
    n_cb = Cout // QB
    scale = 1.0 / math.sqrt(D)
    D1 = D + 1
    ca = (n_ct + 1) // 2  # x c-tiles in chunk a (rest in chunk b)
    cb_n = n_ct - ca

    assert C % P == 0 and CL % P == 0 and T % QB == 0 and Cout % QB == 0
    assert HL % 2 == 0 and D == 64 and n_mt == n_hp

    nc = bacc.Bacc("TRN2", target_bir_lowering=False, debug=False)
    # x first q-block, half-packed for 2KB DMA rows: row r = [x[r] | x[C/2+r]]
    x0T = nc.dram_tensor("x0T", [C // 2, 2 * QB], BF16, kind="ExternalInput")
    xT = nc.dram_tensor("xT", [C, T], BF16, kind="ExternalInput")
    # per-hp k|q chunks, half-packed into 1KB rows the same way
    wqk = nc.dram_tensor("wqk", [n_hp, C // 2, 4 * P], BF16, kind="ExternalInput")
    wv = nc.dram_tensor("wv", [C, CL], BF16, kind="ExternalInput")
    wp = nc.dram_tensor("wp", [CL, Cout], BF16, kind="ExternalInput")
    masks = nc.dram_tensor("masks", [P, P], BF16, kind="ExternalInput")
    out = nc.dram_tensor("out", [T, Cout], BF16, kind="ExternalOutput")

    with tile.TileContext(nc) as tc, ExitStack() as ctx:
        persist = ctx.enter_context(tc.tile_pool(name="persist", bufs=1))
        # PSUM budget (8 banks): scores 2x[128,1024] + AV 1x[65,1024] + u 2x[128,512]
        ps_sc = ctx.enter_context(tc.tile_pool(name="ps_sc", bufs=2, space="PSUM"))
        ps_av = ctx.enter_context(tc.tile_pool(name="ps_av", bufs=1, space="PSUM"))
        ps_u = ctx.enter_context(tc.tile_pool(name="ps_u", bufs=2, space="PSUM"))
        ppool = ctx.enter_context(tc.tile_pool(name="ppool", bufs=6))
        spool = ctx.enter_context(tc.tile_pool(name="spool", bufs=3))
        bcpool = ctx.enter_context(tc.tile_pool(name="bcpool", bufs=6))
        stage = ctx.enter_context(tc.tile_pool(name="stage", bufs=3))
        opool = ctx.enter_context(tc.tile_pool(name="opool", bufs=2))
        dram = ctx.enter_context(tc.tile_pool(name="dram", bufs=1, space="DRAM"))

        # ---- persistent input tiles; fine-grained DMAs for pipelining,
        # spread over the scalar and sync queues by first-use time ----
        x0_sb = persist.tile([P, n_ct, QB], BF16, tag="x0", name="x0")
        TH = T // 2
        xh_sb = [
            [
                persist.tile([P, TH], BF16, tag=f"x{c}_{h}", name=f"x{c}_{h}")
                for h in range(2)
            ]
            for c in range(n_ct)
        ]
        wqk_sb = persist.tile([P, n_hp, n_ct, 2, P], BF16, tag="wqk", name="wqk")
        wv_sb = [
            persist.tile([P, CL], BF16, tag=f"wv{c}", name=f"wv{c}")
            for c in range(n_ct)
        ]
        wp_sb = persist.tile([P, n_mt, Cout], BF16, tag="wp", name="wp")
        trimask = persist.tile([P, P], BF16, tag="trimask", name="trimask")

        hct = n_ct // 2

        def dma_wqk(hp):
            for h in range(2):
                nc.sync.dma_start(
                    wqk_sb[:][:, hp, hct * h : hct * (h + 1), :, :],
                    wqk[hp][:, h * 2 * P : (h + 1) * 2 * P].rearrange(
                        "(i p) (t w) -> p i t w", p=P, w=P
                    ),
                )

        # scalar queue: x0 halves first (unblock qk0+v0), then x first
        # halves and half the x second halves
        for h in range(2):
            nc.scalar.dma_start(
                x0_sb[:][:, hct * h : hct * (h + 1), :],
                x0T[:, h * QB : (h + 1) * QB].rearrange("(i p) c -> p i c", p=P),
            )
        for c in range(n_ct):
            nc.scalar.dma_start(xh_sb[c][0][:], xT[c * P : (c + 1) * P, 0:TH])
        for c in range(n_ct // 2):
            nc.scalar.dma_start(xh_sb[c][1][:], xT[c * P : (c + 1) * P, TH:])
        # sync queue: hp0 q/k weights and wv in the first in-flight window
        # (the ~6-deep DGE window throttles the bulk behind them)
        dma_wqk(0)
        for c in range(n_ct):
            nc.sync.dma_start(wv_sb[c][:], wv[c * P : (c + 1) * P, :])
        for c in range(n_ct // 2, n_ct):
            nc.sync.dma_start(xh_sb[c][1][:], xT[c * P : (c + 1) * P, TH:])
        nc.sync.dma_start(trimask[:], masks[:])
        for hp in range(1, n_hp):
            dma_wqk(hp)
        nc.sync.dma_start(wp_sb[:], wp.rearrange("(i p) n -> p i n", p=P))

        # warm up the PE clock-gate (HAM) while input DMAs are in flight
        warm = persist.tile([P, QB], BF16, tag="warm", name="warm")
        nc.vector.memset(warm[:], 0.0)
        wps = ps_u.tile([P, QB], F32, tag="u", name="uwarm")
        for _ in range(10):
            nc.tensor.matmul(wps[:], warm[:, 0:P], warm[:], start=True, stop=True)

        def xsrc(c, lo, hi):
            if hi <= QB:
                return x0_sb[:][:, c, lo:hi]
            if hi <= TH:
                return xh_sb[c][0][:, lo:hi]
            return xh_sb[c][1][:, lo - TH : hi - TH]

        def wview(hp, t, c):
            return wqk_sb[:][:, hp, c, t, :]

        sums_d = dram.tile([n_hp * n_qb, 2 * QB], F32, tag="sums_d", name="sums_d")
        recips_d = dram.tile(
            [n_hp * n_qb, 2 * QB], F32, tag="recips_d", name="recips_d"
        )

        # ---- v: interleaved ones column per head: v1 [T, HL*(D+1)] ----
        v1_sb = [
            persist.tile([P, HL * D1], BF16, tag=f"v1_{tt}", name=f"v1_{tt}")
            for tt in range(n_tt)
        ]

        def emit_v(tts):
            for tt in tts:
                t = v1_sb[tt]
                ones_view = t[:].rearrange("p (h e) -> p h e", h=HL)[:, :, 0:1]
                nc.vector.memset(ones_view, 1.0)
                ps = ps_u.tile([P, CL], F32, tag="u", name="uv")
                for c in range(n_ct):
                    nc.tensor.matmul(
                        ps[:],
                        xsrc(c, tt * P, (tt + 1) * P),
                        wv_sb[c][:],
                        start=(c == 0),
                        stop=(c == n_ct - 1),
                    )
                dst_view = t[:].rearrange("p (h e) -> p h e", h=HL)[:, :, 1 : D + 1]
                src_view = ps[:].rearrange("p (h e) -> p h e", h=HL)
                nc.vector.tensor_copy(dst_view, src_view)

        # ---- q/k projections (d-major) ----
        yT_sb = [
            persist.tile([P, T], BF16, tag=f"yT{m}", name=f"yT{m}")
            for m in range(n_mt)
        ]
        q_d = [[] for _ in range(n_mt)]
        k_d = [[] for _ in range(n_mt)]

        def emit_qk(hp, bs):
            # ascending contiguous b's per hp
            for b in bs:
                for t, name, dst in ((0, "k", k_d), (1, "q", q_d)):
                    tl = persist.tile(
                        [P, QB], BF16, tag=f"{name}d{hp}_{b}", name=f"{name}d{hp}_{b}"
                    )
                    dst[hp].append(tl)
                    ps = ps_u.tile([P, QB], F32, tag="u", name="uqk")
                    for c in range(n_ct):
                        nc.tensor.matmul(
                            ps[:],
                            wview(hp, t, c),
                            xsrc(c, b * QB, (b + 1) * QB),
                            start=(c == 0),
                            stop=(c == n_ct - 1),
                        )
                    nc.vector.tensor_copy(tl[:], ps[:])

        def emit_attn(hp, qb):
            yts = ps_av.tile([D1, 2 * QB], F32, tag="av", name="yts")
            n_kt = dpb * qb + dpb

            def emit_av(kt, pt):
                q0 = P * max(kt - dpb * qb, 0)
                for i in range(2):
                    h = 2 * hp + i
                    nc.tensor.matmul(
                        yts[:, i * QB + q0 : (i + 1) * QB],
                        v1_sb[kt][:, h * D1 : (h + 1) * D1],
                        pt[:, i * QB + q0 : (i + 1) * QB],
                        start=(kt == 0),
                        stop=(kt == n_kt - 1),
                        skip_group_check=True,
                    )

            pending = []
            for kt in range(n_kt):
                m = kt - dpb * qb  # >=0: diagonal tile index
                s0 = P * max(m, 0)
                st = ps_sc.tile([P, 2 * QB], F32, tag="sc", name="st")
                for i in range(2):
                    base = 64 * i
                    nc.tensor.matmul(
                        st[:, i * QB + s0 : (i + 1) * QB],
                        k_d[hp][kt // dpb][
                            base : base + 64, (kt % dpb) * P : (kt % dpb + 1) * P
                        ],
                        q_d[hp][qb][base : base + 64, s0:],
                        start=True,
                        stop=True,
                    )
                pt = ppool.tile([P, 2 * QB], BF16, tag="pt", name="pt")
                if m <= 0:
                    nc.scalar.activation(
                        pt[:], st[:], mybir.ActivationFunctionType.Exp, scale=scale
                    )
                else:
                    nc.scalar.activation(
                        pt[:].rearrange("p (i q) -> p i q", i=2)[:, :, s0:],
                        st[:].rearrange("p (i q) -> p i q", i=2)[:, :, s0:],
                        mybir.ActivationFunctionType.Exp,
                        scale=scale,
                    )
                if m >= 0:
                    q0 = P * m
                    sl = pt[:].rearrange("p (i q) -> p i q", i=2)[:, :, q0 : q0 + P]
                    nc.gpsimd.tensor_mul(
                        sl, sl, trimask[:, None, :].broadcast_to([P, 2, P])
                    )
                pending.append((kt, pt))
                if len(pending) > 2:
                    emit_av(*pending.pop(0))
            for item in pending:
                emit_av(*item)

            # epilogue: one PSUM->SBUF copy for both heads; write unnormalized
            # y into yT (casting SWDGE, both head halves in one DMA)
            ys = spool.tile([D1, 2 * QB], F32, tag="ys", name="ys")
            rc = None
            if hp == n_hp - 1 and qb == n_qb - 1:
                # tail: land the sum row first (bf16) so the broadcast
                # matmuls can start while the full copy still runs
                rc = stage.tile([2, 2 * QB], BF16, tag="st_rc", name="rc")
                nc.vector.tensor_copy(rc[:], yts[0:2, :])
            nc.vector.tensor_copy(ys[:], yts[:])
            for i in range(2):
                nc.gpsimd.dma_start(
                    yT_sb[hp][64 * i : 64 * i + 64, qb * QB : (qb + 1) * QB],
                    ys[1 : D + 1, i * QB : (i + 1) * QB],
                )
            if hp < n_hp - 1 or qb < n_qb - 1:
                nc.sync.dma_start(
                    sums_d[hp * n_qb + qb : hp * n_qb + qb + 1, :], ys[0:1, :]
                )
            return ys, rc

        bc_tiles = {}  # (hp, qb) -> broadcast reciprocal tile

        def emit_bc(hp, qb):
            r = hp * n_qb + qb
            bc = bcpool.tile([P, QB], F32, tag="bc", name="bc")
            for i in range(2):
                nc.sync.dma_start(
                    bc[64 * i : 64 * i + 64, :],
                    recips_d[r : r + 1, i * QB : (i + 1) * QB].to_broadcast(
                        (64, QB)
                    ),
                )
            bc_tiles[(hp, qb)] = bc

        def emit_norm_chain(hp):
            # hp0-2: batched at head-pair end; latency hidden by next hp
            r0 = hp * n_qb
            allsums = stage.tile([n_qb, 2 * QB], F32, tag="st_s", name="allsums")
            nc.sync.dma_start(allsums[:], sums_d[r0 : r0 + n_qb, :])
            allrec = stage.tile([n_qb, 2 * QB], F32, tag="st_s", name="allrec")
            nc.vector.reciprocal_approx_fast(allrec[:], allsums[:])
            nc.sync.dma_start(recips_d[r0 : r0 + n_qb, :], allrec[:])
            for qb in range(n_qb):
                emit_bc(hp, qb)

        ones64 = persist.tile([1, 64], BF16, tag="ones64", name="ones64")
        nc.vector.memset(ones64[:], 1.0)

        def emit_norm3_chain(qb, ys, rc=None):
            # hp3: per-q-block. reciprocal_approx_fast misbehaves on a
            # 1-partition AP on HW, so first broadcast the ys sum row across
            # partitions. The last q-block is latency-critical (serial tail):
            # use a PE K=1 broadcast matmul + 128-partition reciprocal with
            # no DMA hops; earlier blocks take the (cheaper, hidden) DRAM
            # gather path.
            hp = n_hp - 1
            r = hp * n_qb + qb
            if qb == n_qb - 1:
                sb = ps_u.tile([P, QB], F32, tag="u", name="ubc")
                for i in range(2):
                    nc.tensor.matmul(
                        sb[64 * i : 64 * i + 64, :],
                        ones64[:],
                        rc[0:1, i * QB : (i + 1) * QB],
                        start=True,
                        stop=True,
                    )
                bc = bcpool.tile([P, QB], F32, tag="bc", name="bc")
                nc.vector.reciprocal_approx_fast(bc[:], sb[:])
                bc_tiles[(hp, qb)] = bc
            else:
                two = stage.tile([2, QB], F32, tag="st_r", name="sums2")
                nc.sync.dma_start(
                    two[:], sums_d[r : r + 1, :].rearrange("o (p c) -> (o p) c", p=2)
                )
                rec2 = stage.tile([2, QB], F32, tag="st_r", name="rec2")
                nc.vector.reciprocal_approx_fast(rec2[:], two[:])
                nc.sync.dma_start(
                    recips_d[r : r + 1, :].rearrange("o (p c) -> (o p) c", p=2),
                    rec2[:],
                )
                emit_bc(hp, qb)

        def emit_norm_muls(hp):
            for qb in range(n_qb):
                sl = yT_sb[hp][:, qb * QB : (qb + 1) * QB]
                nc.vector.tensor_mul(sl, sl, bc_tiles[(hp, qb)][:])

        def emit_proj(qb):
            # normalize hp3's block first, then project t-range of this qb
            hp = n_hp - 1
            sl = yT_sb[hp][:, qb * QB : (qb + 1) * QB]
            nc.vector.tensor_mul(sl, sl, bc_tiles[(hp, qb)][:])
            for tt in range(dpb * qb, dpb * (qb + 1)):
                ost = opool.tile([P, n_cb * QB], BF16, tag="ost", name="ost")
                for cb in range(n_cb):
                    ps = ps_u.tile([P, QB], F32, tag="u", name="upj")
                    for m in range(n_mt):
                        nc.tensor.matmul(
                            ps[:],
                            yT_sb[m][:, tt * P : (tt + 1) * P],
                            wp_sb[:][:, m, cb * QB : (cb + 1) * QB],
                            start=(m == 0),
                            stop=(m == n_mt - 1),
                        )
                    nc.vector.tensor_copy(ost[:, cb * QB : (cb + 1) * QB], ps[:])
                nc.sync.dma_start(out[tt * P : (tt + 1) * P, :], ost[:])

        # ---- schedule ----
        emit_qk(0, [0])
        emit_v(range(0, dpb))
        for qb in range(n_qb):
            emit_attn(0, qb)
            if qb < n_qb - 1:
                emit_qk(0, [qb + 1])
                emit_v(range(dpb * (qb + 1), dpb * (qb + 2)))
            emit_qk(1, [qb])
        emit_norm_chain(0)
        for hp in range(1, n_hp):
            for qb in range(n_qb):
                ys_last, rc_last = emit_attn(hp, qb)
                if hp < n_hp - 2:
                    emit_qk(hp + 1, [qb])
                elif hp == n_hp - 2:
                    if qb < n_qb - 1:
                        emit_qk(hp + 1, [qb])
                else:
                    if qb == 0:
                        emit_qk(hp, [n_qb - 1])
                    emit_norm3_chain(qb, ys_last, rc_last)
                    if qb > 0:
                        emit_proj(qb - 1)
                if qb == 0:
                    emit_norm_muls(hp - 1)
            if hp < n_hp - 1:
                emit_norm_chain(hp)
        emit_proj(n_qb - 1)

    nc.compile()
    return nc


_PROGRAM_CACHE = {}


def _get_program(C_eff):
    key = C_eff
    if key not in _PROGRAM_CACHE:
        _PROGRAM_CACHE[key] = build_kernel(T=T, C=C_eff, HL=HL, D=D, Cout=C)
    return _PROGRAM_CACHE[key]


def _make_in_maps(x, Wqkv, bqkv):
    """Shard + cast inputs for the 8 cores. Returns (in_maps, C_eff)."""
    if np.any(bqkv):
        # Fold the qkv bias in as an extra contraction row (x gains a ones
        # column), zero-padded up to a multiple of 128.
        C_eff = ((C + 1 + P - 1) // P) * P
        Waug = np.zeros((C_eff, 3 * C), dtype=np.float32)
        Waug[:C] = Wqkv
        Waug[C] = bqkv
    else:
        C_eff = C
        Waug = Wqkv

    n_hp = HL // 2
    masks = (np.arange(P)[:, None] <= np.arange(P)[None, :]).astype(NPBF16)
    in_maps = []
    for core in range(N_CORES):
        b, hh = divmod(core, N_CORES // B)
        xT = np.zeros((C_eff, T), dtype=np.float32)
        xT[:C] = x[b].T
        if C_eff > C:
            xT[C] = 1.0
        c0 = hh * CL
        # wqk packed hp-major with k|q side by side: [hp, C_eff, 2*128]
        wqk = np.empty((n_hp, C_eff, 2 * P), dtype=np.float32)
        for hp in range(n_hp):
            wqk[hp, :, :P] = Waug[:, 1 * C + c0 + hp * P : 1 * C + c0 + (hp + 1) * P]
            wqk[hp, :, P:] = Waug[:, 0 * C + c0 + hp * P : 0 * C + c0 + (hp + 1) * P]
        wqkp = np.concatenate([wqk[:, : C_eff // 2], wqk[:, C_eff // 2 :]], axis=2)
        xTb = xT.astype(NPBF16)
        x0p = np.concatenate(
            [xTb[: C_eff // 2, :QB], xTb[C_eff // 2 :, :QB]], axis=1
        )
        in_maps.append(
            {
                "x0T": np.ascontiguousarray(x0p),
                "xT": xTb,
                "wqk": np.ascontiguousarray(wqkp).astype(NPBF16),
                "wv": np.ascontiguousarray(
                    Waug[:, 2 * C + c0 : 2 * C + c0 + CL]
                ).astype(NPBF16),
                "wp": None,  # filled below (depends only on hh)
                "masks": masks,
            }
        )
    return in_maps, C_eff


def _run(x, Wqkv, bqkv, Wproj, bproj, trace=False):
    from concourse.bass_utils import run_bass_kernel_spmd

    in_maps, C_eff = _make_in_maps(x, Wqkv, bqkv)
    wp_by_hh = [
        np.ascontiguousarray(Wproj[hh * CL : (hh + 1) * CL, :]).astype(NPBF16)
        for hh in range(N_CORES // B)
    ]
    for core in range(N_CORES):
        in_maps[core]["wp"] = wp_by_hh[core % (N_CORES // B)]

    nc = _get_program(C_eff)
    res = run_bass_kernel_spmd(
        nc, in_maps, core_ids=list(range(N_CORES)), trace=trace
    )

    halves = N_CORES // B
    y = np.empty((B, T, C), dtype=np.float32)
    for b in range(B):
        acc = res.results[b * halves]["out"].astype(np.float32)
        for hh in range(1, halves):
            acc = acc + res.results[b * halves + hh]["out"].astype(np.float32)
        y[b] = acc + bproj.astype(np.float32)
    return y, res


def kernel(x, Wqkv, bqkv, Wproj, bproj):
    y, _ = _run(
        np.asarray(x, dtype=np.float32),
        np.asarray(Wqkv, dtype=np.float32),
        np.asarray(bqkv, dtype=np.float32),
        np.asarray(Wproj, dtype=np.float32),
        np.asarray(bproj, dtype=np.float32),
        trace=False,
    )
    return y


if __name__ == "__main__":
    # quick compile check
    build_kernel()
    print("compile OK")
